# revision 7
# baseline (speedup 1.0000x reference)
"""Trainium2 Bass kernel for nn_ExpertRouter (dense MoE routing).

Reference computation (per token t of T=4096, D=6144, MID=512, NE=16):
    h[t,n,:] = relu(xf[t] @ w1[n] + b1[n])          # [T, NE, MID]
    e[t,n]   = h[t,n] . w2[n] + b2[n]               # [T, NE]
    g[t,:]   = softmax(xf[t] @ gw + gb)             # [T, NE]
    out[t]   = sigmoid(sum_n g[t,n] * e[t,n])

Strategy: data-parallel over tokens across 8 NeuronCores (512 tokens/core,
weights replicated, no collectives). Dominant compute = 16 expert matmuls
[512,6144]@[6144,512] per core in bf16 with fp32 PSUM accumulation.
Softmax division is deferred: out = sigmoid((sum_n expl[n]*(e_n+b2)) / sum_n expl[n])
so no per-expert normalization is needed and exp() is computed without
max-subtraction (logit std ~0.58, safe in fp32).

All inputs are rearranged on the host into layouts that make every DMA
fully contiguous per partition.
"""

import numpy as np
import ml_dtypes

# problem constants (hardcoded per harness contract)
B, NW, WS, FD = 16, 256, 8, 96
D = WS * WS * FD          # 6144
MID = 512
NE = 16
T = B * NW                # 4096 tokens
NCORES = 8
TOK = T // NCORES         # 512 tokens per core
P = 128                   # partitions
KT = D // P               # 48 contraction tiles
MT = MID // P             # 4 mid tiles

_CACHE = {}


import os

USE_FP8 = os.environ.get("ER_FP8", "1") == "1"  # fp8-e4m3 DoubleRow big matmuls
# drsw: software-interleaved weights (contiguous LDWEIGHTS read)
FP8_MODE = os.environ.get("ER_MODE", "dr")
FP8_SCALE = 128.0  # w1 pre-scale so U(-1/sqrt(D),..) lands in e4m3 normal range
KT2 = KT // 2      # DoubleRow k-steps (2 contraction rows per partition)


def _build(reps=1, use_fp8=USE_FP8):
    """Build + compile the per-core SPMD bass program. Returns nc.

    reps>1 wraps the whole body in a Tile For loop — used only for
    slope-based HW timing (fixed dispatch overhead cancels between rep
    counts); the graded kernel uses reps=1 (no loop)."""
    import contextlib
    import concourse.tile as tile
    from concourse import bacc, mybir

    bf16 = mybir.dt.bfloat16
    fp8 = mybir.dt.float8e4
    f32 = mybir.dt.float32
    AF = mybir.ActivationFunctionType
    ALU = mybir.AluOpType

    nc = bacc.Bacc("TRN2", target_bir_lowering=False, debug=False)

    drsw = use_fp8 and FP8_MODE == "drsw"
    pm = (
        mybir.MatmulPerfMode.DoubleRowSwInterleave
        if drsw
        else mybir.MatmulPerfMode.DoubleRow
    )
    xT_d = nc.dram_tensor("xT", [P, KT, TOK], bf16, kind="ExternalInput").ap()
    if use_fp8:
        xq_d = nc.dram_tensor("xq", [P, KT2, 2, TOK], fp8, kind="ExternalInput").ap()
        if drsw:
            # per-(k2, mt) stationary chunk pre-interleaved: [.., j, i] =
            # w1[n, k2*256 + i*128 + p, mt*128 + (127-j)] (A/B pairs, cols
            # reversed) so LDWEIGHTS reads contiguously
            w1_d = nc.dram_tensor(
                "w1", [NE, P, KT2, MT, P, 2], fp8, kind="ExternalInput"
            ).ap()
        else:
            w1_d = nc.dram_tensor(
                "w1", [NE, P, KT2, 2, MID], fp8, kind="ExternalInput"
            ).ap()
    else:
        # mt-major layout so each [P, KT, 128] mid-chunk is one contiguous DMA
        w1_d = nc.dram_tensor(
            "w1", [NE, MT, P, KT, P], bf16, kind="ExternalInput"
        ).ap()
    gw_d = nc.dram_tensor("gw", [P, KT, NE], bf16, kind="ExternalInput").ap()
    b1_d = nc.dram_tensor("b1", [P, NE, MT], f32, kind="ExternalInput").ap()
    w2_d = nc.dram_tensor("w2", [P, NE, MT], bf16, kind="ExternalInput").ap()
    b2_d = nc.dram_tensor("b2", [1, NE], f32, kind="ExternalInput").ap()
    gb_d = nc.dram_tensor("gb", [NE, 1], f32, kind="ExternalInput").ap()
    out_d = nc.dram_tensor("out", [1, TOK], f32, kind="ExternalOutput").ap()

    with tile.TileContext(nc) as tc:
        loop_ctx = (
            tc.For_i(0, reps, 1) if reps > 1 else contextlib.nullcontext()
        )
        with (
            loop_ctx,
            tc.tile_pool(name="consts", bufs=1) as consts,
            tc.tile_pool(name="xpool", bufs=1) as xpool,
            tc.tile_pool(name="wpool", bufs=2) as wpool,
            tc.tile_pool(name="hpool", bufs=4) as hpool,
            tc.tile_pool(name="small", bufs=2) as small,
            tc.tile_pool(name="acc", bufs=1) as accp,
            tc.tile_pool(name="ps_h", bufs=2, space="PSUM") as ps_h,
            tc.tile_pool(name="ps_g", bufs=1, space="PSUM") as ps_g,
            tc.tile_pool(name="ps_e", bufs=2, space="PSUM") as ps_e,
        ):
            # resident inputs; xT DMA'd in 8 k-chunks so gating (and the
            # first expert) can start as soon as early chunks land
            XCH = 6
            xT = xpool.tile([P, KT, TOK], bf16)
            for c in range(KT // XCH):
                nc.sync.dma_start(
                    xT[:, c * XCH:(c + 1) * XCH, :], xT_d[:, c * XCH:(c + 1) * XCH, :]
                )
            if use_fp8:
                xq = xpool.tile([P, KT2, 2, TOK], fp8)
                nc.sync.dma_start(xq[:], xq_d[:])
            gw = consts.tile([P, KT, NE], bf16)
            nc.sync.dma_start(gw[:], gw_d[:])
            b1 = consts.tile([P, NE, MT], f32)
            nc.sync.dma_start(b1[:], b1_d[:])
            w2 = consts.tile([P, NE, MT], bf16)
            nc.sync.dma_start(w2[:], w2_d[:])
            b2 = consts.tile([1, NE], f32)
            nc.sync.dma_start(b2[:], b2_d[:])
            gb = consts.tile([NE, 1], f32)
            nc.sync.dma_start(gb[:], gb_d[:])
            ones = consts.tile([NE, 1], f32)
            nc.vector.memset(ones[:], 1.0)

            # gating logits: gl[e, t] = sum_d gw[d, e] * x[d, t]
            gl = ps_g.tile([NE, TOK], f32)
            for k in range(KT):
                nc.tensor.matmul(
                    gl[:], gw[:, k, :], xT[:, k, :], start=(k == 0), stop=(k == KT - 1)
                )
            expl = consts.tile([NE, TOK], f32)
            nc.scalar.activation(expl[:], gl[:], AF.Exp, bias=gb[:])

            # denominator: den[t] = sum_e expl[e, t]
            den = ps_g.tile([1, TOK], f32)
            nc.tensor.matmul(den[:], ones[:], expl[:], start=True, stop=True)
            rec = consts.tile([1, TOK], f32)
            nc.vector.reciprocal(rec[:], den[:])

            # flatten expl rows onto partition 0 so per-expert weighting is
            # a partition-0 elementwise op (cross-partition move via DMA)
            explf = consts.tile([1, NE * TOK], f32)
            for n in range(NE):
                nc.sync.dma_start(explf[0:1, n * TOK:(n + 1) * TOK], expl[n:n + 1, :])

            # weighted-sum accumulator on partition 0
            u = accp.tile([1, TOK], f32)
            nc.vector.memset(u[:], 0.0)

            for n in range(NE):
                if use_fp8:
                    w1s = wpool.tile(
                        [P, KT2, MT, P, 2] if drsw else [P, KT2, 2, MID], fp8
                    )
                    nc.sync.dma_start(w1s[:], w1_d[n])
                e_ps = ps_e.tile([1, TOK], f32)
                for mt in range(MT):
                    if not use_fp8:
                        # one [P, KT, 128] weight chunk per mid-tile: finer
                        # prefetch granularity, smaller SBUF footprint
                        w1c = wpool.tile([P, KT, P], bf16, bufs=8)
                        nc.sync.dma_start(w1c[:], w1_d[n, mt])
                    h_ps = ps_h.tile([P, TOK], f32)
                    if use_fp8:
                        for k2 in range(KT2):
                            nc.tensor.matmul(
                                h_ps[:],
                                w1s[:, k2, mt, :, :]
                                if drsw
                                else w1s[:, k2, :, mt * P:(mt + 1) * P],
                                xq[:, k2, :, :],
                                start=(k2 == 0),
                                stop=(k2 == KT2 - 1),
                                perf_mode=pm,
                            )
                    else:
                        for k in range(KT):
                            nc.tensor.matmul(
                                h_ps[:],
                                w1c[:, k, :],
                                xT[:, k, :],
                                start=(k == 0),
                                stop=(k == KT - 1),
                            )
                    h_sb = hpool.tile([P, TOK], bf16)
                    nc.scalar.activation(
                        h_sb[:],
                        h_ps[:],
                        AF.Relu,
                        bias=b1[:, n, mt:mt + 1],
                        scale=(1.0 / FP8_SCALE) if use_fp8 else 1.0,
                    )
                    nc.tensor.matmul(
                        e_ps[:],
                        w2[:, n, mt:mt + 1],
                        h_sb[:],
                        start=(mt == 0),
                        stop=(mt == MT - 1),
                        skip_group_check=True,
                    )
                # u += (e + b2[n]) * expl[n]
                tmp = small.tile([1, TOK], f32)
                nc.vector.scalar_tensor_tensor(
                    tmp[:],
                    e_ps[:],
                    b2[0:1, n:n + 1],
                    explf[0:1, n * TOK:(n + 1) * TOK],
                    ALU.add,
                    ALU.mult,
                )
                nc.vector.tensor_add(u[:], u[:], tmp[:])

            # out = sigmoid(u / den)
            s = small.tile([1, TOK], f32)
            nc.vector.tensor_mul(s[:], u[:], rec[:])
            o = small.tile([1, TOK], f32)
            nc.scalar.activation(o[:], s[:], AF.Sigmoid)
            nc.sync.dma_start(out_d[:], o[:])

    nc.compile()
    return nc


def _prep_inputs(x, w1, b1, w2, b2, gw, gb, use_fp8=USE_FP8):
    """Host-side shard + layout prep. Returns per-core in_maps."""
    bf = ml_dtypes.bfloat16
    import concourse.mybir as mybir

    fp8np = mybir.dt.np(mybir.dt.float8e4)
    xf = np.ascontiguousarray(np.asarray(x, np.float32)).reshape(T, D)
    # xT[core][p, k, t] = xf[core*TOK + t, k*P + p]
    xp = xf.reshape(NCORES, TOK, KT, P).transpose(0, 3, 2, 1).astype(bf)
    if use_fp8:
        # xq[core][p, k2, h, t] = xf[core*TOK + t, k2*256 + h*128 + p]
        xqp = (
            xf.reshape(NCORES, TOK, KT2, 2, P)
            .transpose(0, 4, 2, 3, 1)
            .astype(fp8np)
        )
        if FP8_MODE == "drsw":
            # A[n, k2, h, p, mt, m]
            a = (np.asarray(w1, np.float32) * FP8_SCALE).reshape(
                NE, KT2, 2, P, MT, P
            )
            # [n, p, k2, mt, j, i] with j = reversed m, i = row-half
            w1p = np.ascontiguousarray(
                a[:, :, :, :, :, ::-1].transpose(0, 3, 1, 4, 5, 2).astype(fp8np)
            )
        else:
            # w1q[n, p, k2, h, m] = FP8_SCALE * w1[n, k2*256 + h*128 + p, m]
            w1p = np.ascontiguousarray(
                (np.asarray(w1, np.float32) * FP8_SCALE)
                .reshape(NE, KT2, 2, P, MID)
                .transpose(0, 3, 1, 2, 4)
                .astype(fp8np)
            )
    else:
        # [NE, MT, P, KT, P]: w1p[n, mt, p, k, m] = w1[n, k*P+p, mt*P+m]
        w1p = np.ascontiguousarray(
            np.asarray(w1, np.float32)
            .reshape(NE, KT, P, MT, P)
            .transpose(0, 3, 2, 1, 4)
            .astype(bf)
        )
    gwp = np.ascontiguousarray(
        np.asarray(gw, np.float32).reshape(KT, P, NE).transpose(1, 0, 2).astype(bf)
    )
    b1p = np.ascontiguousarray(
        np.asarray(b1, np.float32).reshape(NE, MT, P).transpose(2, 0, 1)
    )
    w2p = np.ascontiguousarray(
        np.asarray(w2, np.float32).reshape(NE, MT, P).transpose(2, 0, 1).astype(bf)
    )
    b2p = np.asarray(b2, np.float32).reshape(1, NE)
    gbp = np.asarray(gb, np.float32).reshape(NE, 1)

    in_maps = []
    for c in range(NCORES):
        m = {
            "xT": np.ascontiguousarray(xp[c]),
            "w1": w1p,
            "gw": gwp,
            "b1": b1p,
            "w2": w2p,
            "b2": b2p,
            "gb": gbp,
        }
        if use_fp8:
            m["xq"] = np.ascontiguousarray(xqp[c])
        in_maps.append(m)
    return in_maps


def kernel(x, w1, b1, w2, b2, gw, gb):
    from concourse import bass_utils

    if "nc" not in _CACHE:
        _CACHE["nc"] = _build()
    nc = _CACHE["nc"]
    in_maps = _prep_inputs(x, w1, b1, w2, b2, gw, gb)
    res = bass_utils.run_bass_kernel_spmd(nc, in_maps, core_ids=list(range(NCORES)))
    out = np.concatenate([r["out"].reshape(TOK) for r in res.results])
    return out.reshape(B, NW).astype(np.float32)



# revision 14
# speedup vs baseline: 1.3588x; 1.3588x over previous
"""Trainium2 Bass kernel for nn_ExpertRouter (dense MoE routing).

Reference computation (per token t of T=4096, D=6144, MID=512, NE=16):
    h[t,n,:] = relu(xf[t] @ w1[n] + b1[n])          # [T, NE, MID]
    e[t,n]   = h[t,n] . w2[n] + b2[n]               # [T, NE]
    g[t,:]   = softmax(xf[t] @ gw + gb)             # [T, NE]
    out[t]   = sigmoid(sum_n g[t,n] * e[t,n])

Strategy: data-parallel over tokens across 8 NeuronCores (512 tokens/core,
weights replicated, no collectives). Dominant compute = 16 expert matmuls
[512,6144]@[6144,512] per core in bf16 with fp32 PSUM accumulation.
Softmax division is deferred: out = sigmoid((sum_n expl[n]*(e_n+b2)) / sum_n expl[n])
so no per-expert normalization is needed and exp() is computed without
max-subtraction (logit std ~0.58, safe in fp32).

All inputs are rearranged on the host into layouts that make every DMA
fully contiguous per partition.
"""

import numpy as np
import ml_dtypes

# problem constants (hardcoded per harness contract)
B, NW, WS, FD = 16, 256, 8, 96
D = WS * WS * FD          # 6144
MID = 512
NE = 16
T = B * NW                # 4096 tokens
NCORES = 8
TOK = T // NCORES         # 512 tokens per core
P = 128                   # partitions
KT = D // P               # 48 contraction tiles
MT = MID // P             # 4 mid tiles

_CACHE = {}


import os

# "ws": weight-stationary (per-MM LDWEIGHTS), "xs": x-stationary (one
# LDWEIGHTS per token-block/k-step shared by all expert matmuls)
IMPL = os.environ.get("ER_IMPL", "ws")

USE_FP8 = os.environ.get("ER_FP8", "1") == "1"  # fp8-e4m3 DoubleRow big matmuls
# drsw: software-interleaved weights (contiguous LDWEIGHTS read)
FP8_MODE = os.environ.get("ER_MODE", "dr")
FP8_SCALE = 128.0  # w1 pre-scale so U(-1/sqrt(D),..) lands in e4m3 normal range
KT2 = KT // 2      # DoubleRow k-steps (2 contraction rows per partition)


def _ldw_sig(inst):
    a = inst.ins[0]
    return (
        a.memref,
        a.offset,
        str(a.ap),
        str(a.dtype),
        str(getattr(inst, "perf_mode", None)),
        str(getattr(inst, "tile_position", None)),
        str(getattr(inst, "is_transpose", None)),
    )


def _dedupe_ldweights(nc):
    """Drop InstLdweights whose stationary operand is identical to the
    previously retained load on PE (walrus matmuls are non-self-loading, so
    the array still holds those weights). Only sync-free loads are removed."""
    removed = 0
    for fn in nc.m.functions:
        for blk in fn.blocks:
            keep = []
            last_sig = None
            for inst in blk.instructions:
                tn = type(inst).__name__
                if tn == "InstLdweights":
                    si = inst.sync_info
                    clean = si is None or (
                        len(si.on_wait) == 0 and len(si.on_update) == 0
                    )
                    sig = _ldw_sig(inst)
                    if clean and sig == last_sig:
                        removed += 1
                        continue
                    last_sig = sig
                elif tn == "InstMatmult":
                    if getattr(inst, "is_transpose", None):
                        last_sig = None
            # any other engine's instructions don't touch PE weights
                keep.append(inst)
            if len(keep) != len(blk.instructions):
                try:
                    blk.instructions[:] = keep
                except TypeError:
                    blk.instructions = keep
    return removed


# x-stationary layout constants
K2C = KT2          # 24 real contraction pairs
K2B = K2C + 1      # +1 bias pair (ones row in x, b1 rows in w1)
NTB = TOK // P     # 4 token blocks of 128
NGRP = 4           # experts per pass (PSUM/SBUF limited)
NPASS = NE // NGRP # 4 passes


def _build_xs(reps=1):
    """x-stationary fp8 DoubleRow kernel: stationary = x token-block chunk
    (one LDWEIGHTS per (pass, tb, k2), deduped across the NGRP expert matmuls
    + gating matmul that share it), moving = w1 slices [P, 2, MID].

    h lands as [token, mid] in PSUM; e = relu(h)·w2 via one fused DVE
    scalar_tensor_tensor with accum_out; gating logits land as [token,
    expert] directly, softmax/gated-sum are free-dim DVE ops."""
    import contextlib
    import concourse.tile as tile
    from concourse import bacc, mybir

    bf16 = mybir.dt.bfloat16
    fp8 = mybir.dt.float8e4
    f32 = mybir.dt.float32
    AF = mybir.ActivationFunctionType
    ALU = mybir.AluOpType
    DR = mybir.MatmulPerfMode.DoubleRow

    nc = bacc.Bacc("TRN2", target_bir_lowering=False, debug=False)

    xq_d = nc.dram_tensor("xq", [P, K2B, 2, TOK], fp8, kind="ExternalInput").ap()
    w1_d = nc.dram_tensor(
        "w1", [NPASS, K2B, P, NGRP, 2, MID], fp8, kind="ExternalInput"
    ).ap()
    gwq_d = nc.dram_tensor("gwq", [P, K2B, 2, NE], fp8, kind="ExternalInput").ap()
    w2b_d = nc.dram_tensor("w2b", [P, NE, MID], bf16, kind="ExternalInput").ap()
    b2b_d = nc.dram_tensor("b2b", [P, NE], f32, kind="ExternalInput").ap()
    out_d = nc.dram_tensor("out", [P, NTB], f32, kind="ExternalOutput").ap()

    with tile.TileContext(nc) as tc:
        loop_ctx = tc.For_i(0, reps, 1) if reps > 1 else contextlib.nullcontext()
        with (
            loop_ctx,
            tc.tile_pool(name="consts", bufs=1) as consts,
            tc.tile_pool(name="xpool", bufs=1) as xpool,
            tc.tile_pool(name="wpool", bufs=K2B) as wpool,
            tc.tile_pool(name="etp", bufs=3) as etp,
            tc.tile_pool(name="small", bufs=2) as small,
            tc.tile_pool(name="ps_h", bufs=6, space="PSUM") as ps_h,
            tc.tile_pool(name="ps_g", bufs=2, space="PSUM") as ps_g,
        ):
            xq = xpool.tile([P, K2B, 2, TOK], fp8)
            XC = 5
            for c in range(K2B // XC):
                nc.sync.dma_start(
                    xq[:, c * XC:(c + 1) * XC], xq_d[:, c * XC:(c + 1) * XC]
                )
            gwq = consts.tile([P, K2B, 2, NE], fp8)
            nc.sync.dma_start(gwq[:], gwq_d[:])
            w2b = consts.tile([P, NE, MID], bf16)
            nc.sync.dma_start(w2b[:], w2b_d[:])
            b2b = consts.tile([P, NE], f32)
            nc.sync.dma_start(b2b[:], b2b_d[:])
            e_all = consts.tile([P, NTB * NE], f32)
            explT = consts.tile([P, NTB, NE], bf16)
            out_sb = consts.tile([P, NTB], f32)

            for ps in range(NPASS):
                wks = []
                for k2 in range(K2B):
                    wk = wpool.tile(
                        [P, NGRP, 2, MID], fp8, name=f"w1k_{ps}_{k2}", tag="w1k"
                    )
                    nc.sync.dma_start(wk[:], w1_d[ps, k2])
                    wks.append(wk)
                for tb in range(NTB):
                    hts = [
                        ps_h.tile([P, MID], f32, name=f"h_{ps}_{tb}_{n}", tag="h")
                        for n in range(NGRP)
                    ]
                    if ps == 0:
                        gl = ps_g.tile([P, NE], f32, name=f"gl_{tb}", tag="gl")
                    for k2 in range(K2B):
                        xsl = xq[:, k2, :, tb * P:(tb + 1) * P]
                        for n in range(NGRP):
                            nc.tensor.matmul(
                                hts[n][:],
                                xsl,
                                wks[k2][:, n, :, :],
                                start=(k2 == 0),
                                stop=(k2 == K2B - 1),
                                perf_mode=DR,
                                skip_group_check=True,
                            )
                        if ps == 0:
                            nc.tensor.matmul(
                                gl[:],
                                xsl,
                                gwq[:, k2, :, :],
                                start=(k2 == 0),
                                stop=(k2 == K2B - 1),
                                perf_mode=DR,
                                skip_group_check=True,
                            )
                    if ps == 0:
                        # g = exp(logits); logits scaled by FP8_SCALE
                        nc.scalar.activation(
                            explT[:, tb, :], gl[:], AF.Exp, scale=1.0 / FP8_SCALE
                        )
                    for n in range(NGRP):
                        ng = ps * NGRP + n
                        et = etp.tile([P, MID], bf16, name=f"et_{ps}_{tb}_{n}",
                                      tag="et")
                        # e[t] = sum_m relu(h[t,m]) * (w2[m]/FP8_SCALE)
                        nc.vector.scalar_tensor_tensor(
                            et[:],
                            hts[n][:],
                            0.0,
                            w2b[:, ng, :],
                            ALU.max,
                            ALU.mult,
                        )
                        nc.vector.tensor_reduce(
                            e_all[:, tb * NE + ng:tb * NE + ng + 1],
                            et[:],
                            mybir.AxisListType.X,
                            ALU.add,
                        )

            for tb in range(NTB):
                t1 = small.tile([P, NE], f32, name=f"t1_{tb}", tag="t1")
                nc.vector.tensor_add(
                    t1[:], e_all[:, tb * NE:(tb + 1) * NE], b2b[:]
                )
                t2 = small.tile([P, NE], f32, name=f"t2_{tb}", tag="t2")
                u_t = small.tile([P, 1], f32, name=f"u_{tb}", tag="u")
                nc.vector.tensor_mul(t2[:], t1[:], explT[:, tb, :])
                nc.vector.tensor_reduce(
                    u_t[:], t2[:], mybir.AxisListType.X, ALU.add
                )
                den = small.tile([P, 1], f32, name=f"den_{tb}", tag="den")
                nc.vector.tensor_reduce(
                    den[:], explT[:, tb, :], mybir.AxisListType.X, ALU.add
                )
                rec = small.tile([P, 1], f32, name=f"rec_{tb}", tag="rec")
                nc.vector.reciprocal(rec[:], den[:])
                pr = small.tile([P, 1], f32, name=f"pr_{tb}", tag="pr")
                nc.vector.tensor_mul(pr[:], u_t[:], rec[:])
                nc.scalar.activation(out_sb[:, tb:tb + 1], pr[:], AF.Sigmoid)
            nc.sync.dma_start(out_d[:], out_sb[:])

    nc.compile()
    if os.environ.get("XS_DEDUPE", "1") == "1":
        n = _dedupe_ldweights(nc)
        assert n > 0, "expected duplicate LDWEIGHTS to remove"
    return nc


def _prep_inputs_xs(x, w1, b1, w2, b2, gw, gb):
    bf = ml_dtypes.bfloat16
    import concourse.mybir as mybir

    fp8np = mybir.dt.np(mybir.dt.float8e4)
    xf = np.ascontiguousarray(np.asarray(x, np.float32)).reshape(T, D)
    # xq[c][p, k2, h, t] = xf[c*TOK + t, k2*256 + h*128 + p]
    xqm = xf.reshape(NCORES, TOK, K2C, 2, P).transpose(0, 4, 2, 3, 1)
    xbias = np.zeros((NCORES, P, 1, 2, TOK), np.float32)
    xbias[:, 0, 0, 0, :] = 1.0
    xqp = np.concatenate([xqm, xbias], axis=2).astype(fp8np)

    # w1p[ps, k2, p, n4, h, m] = S * w1[ps*NGRP+n4, k2*256 + h*128 + p, m]
    a = (np.asarray(w1, np.float32) * FP8_SCALE).reshape(NE, K2C, 2, P, MID)
    w1m = a.reshape(NPASS, NGRP, K2C, 2, P, MID).transpose(0, 2, 4, 1, 3, 5)
    w1bias = np.zeros((NPASS, 1, P, NGRP, 2, MID), np.float32)
    w1bias[:, 0, 0, :, 0, :] = (
        np.asarray(b1, np.float32).reshape(NPASS, NGRP, MID) * FP8_SCALE
    )
    w1p = np.ascontiguousarray(
        np.concatenate([w1m, w1bias], axis=1).astype(fp8np)
    )

    g = (np.asarray(gw, np.float32) * FP8_SCALE).reshape(K2C, 2, P, NE)
    gm = g.transpose(2, 0, 1, 3)  # [p, k2, h, j]
    gbias = np.zeros((P, 1, 2, NE), np.float32)
    gbias[0, 0, 0, :] = np.asarray(gb, np.float32) * FP8_SCALE
    gwqp = np.ascontiguousarray(
        np.concatenate([gm, gbias], axis=1).astype(fp8np)
    )

    w2b = np.ascontiguousarray(
        np.broadcast_to(
            (np.asarray(w2, np.float32) / FP8_SCALE).astype(bf)[None], (P, NE, MID)
        )
    )
    b2b = np.ascontiguousarray(
        np.broadcast_to(np.asarray(b2, np.float32)[None], (P, NE))
    )

    in_maps = []
    for c in range(NCORES):
        in_maps.append(
            {
                "xq": np.ascontiguousarray(xqp[c]),
                "w1": w1p,
                "gwq": gwqp,
                "w2b": w2b,
                "b2b": b2b,
            }
        )
    return in_maps


def _build(reps=1, use_fp8=USE_FP8):
    if IMPL == "xs":
        return _build_xs(reps=reps)
    return _build_ws(reps=reps, use_fp8=use_fp8)


def _build_ws(reps=1, use_fp8=USE_FP8):
    """Build + compile the per-core SPMD bass program. Returns nc.

    reps>1 wraps the whole body in a Tile For loop — used only for
    slope-based HW timing (fixed dispatch overhead cancels between rep
    counts); the graded kernel uses reps=1 (no loop)."""
    import contextlib
    import concourse.tile as tile
    from concourse import bacc, mybir

    bf16 = mybir.dt.bfloat16
    fp8 = mybir.dt.float8e4
    f32 = mybir.dt.float32
    AF = mybir.ActivationFunctionType
    ALU = mybir.AluOpType

    nc = bacc.Bacc("TRN2", target_bir_lowering=False, debug=False)

    drsw = use_fp8 and FP8_MODE == "drsw"
    pm = (
        mybir.MatmulPerfMode.DoubleRowSwInterleave
        if drsw
        else mybir.MatmulPerfMode.DoubleRow
    )
    xT_d = nc.dram_tensor("xT", [P, KT, TOK], bf16, kind="ExternalInput").ap()
    if use_fp8:
        xq_d = nc.dram_tensor("xq", [P, KT2, 2, TOK], fp8, kind="ExternalInput").ap()
        if drsw:
            # per-(k2, mt) stationary chunk pre-interleaved: [.., j, i] =
            # w1[n, k2*256 + i*128 + p, mt*128 + (127-j)] (A/B pairs, cols
            # reversed) so LDWEIGHTS reads contiguously
            w1_d = nc.dram_tensor(
                "w1", [NE, P, KT2, MT, P, 2], fp8, kind="ExternalInput"
            ).ap()
        else:
            w1_d = nc.dram_tensor(
                "w1", [NE, P, KT2, 2, MID], fp8, kind="ExternalInput"
            ).ap()
    else:
        # mt-major layout so each [P, KT, 128] mid-chunk is one contiguous DMA
        w1_d = nc.dram_tensor(
            "w1", [NE, MT, P, KT, P], bf16, kind="ExternalInput"
        ).ap()
    gw_d = nc.dram_tensor("gw", [P, KT, NE], bf16, kind="ExternalInput").ap()
    b1_d = nc.dram_tensor("b1", [P, NE, MT], f32, kind="ExternalInput").ap()
    w2_d = nc.dram_tensor("w2", [P, NE, MT], bf16, kind="ExternalInput").ap()
    b2_d = nc.dram_tensor("b2", [1, NE], f32, kind="ExternalInput").ap()
    gb_d = nc.dram_tensor("gb", [NE, 1], f32, kind="ExternalInput").ap()
    out_d = nc.dram_tensor("out", [1, TOK], f32, kind="ExternalOutput").ap()

    with tile.TileContext(nc) as tc:
        loop_ctx = (
            tc.For_i(0, reps, 1) if reps > 1 else contextlib.nullcontext()
        )
        with (
            loop_ctx,
            tc.tile_pool(name="consts", bufs=1) as consts,
            tc.tile_pool(name="xpool", bufs=1) as xpool,
            tc.tile_pool(name="wpool", bufs=2) as wpool,
            tc.tile_pool(name="hpool", bufs=4) as hpool,
            tc.tile_pool(name="small", bufs=2) as small,
            tc.tile_pool(name="acc", bufs=1) as accp,
            tc.tile_pool(name="ps_h", bufs=2, space="PSUM") as ps_h,
            tc.tile_pool(name="ps_g", bufs=1, space="PSUM") as ps_g,
            tc.tile_pool(name="ps_e", bufs=2, space="PSUM") as ps_e,
        ):
            # resident inputs; xT DMA'd in 8 k-chunks so gating (and the
            # first expert) can start as soon as early chunks land
            XCH = 6
            xT = xpool.tile([P, KT, TOK], bf16)
            for c in range(KT // XCH):
                nc.sync.dma_start(
                    xT[:, c * XCH:(c + 1) * XCH, :], xT_d[:, c * XCH:(c + 1) * XCH, :]
                )
            if use_fp8:
                xq = xpool.tile([P, KT2, 2, TOK], fp8)
                nc.sync.dma_start(xq[:], xq_d[:])
            gw = consts.tile([P, KT, NE], bf16)
            nc.sync.dma_start(gw[:], gw_d[:])
            b1 = consts.tile([P, NE, MT], f32)
            nc.sync.dma_start(b1[:], b1_d[:])
            w2 = consts.tile([P, NE, MT], bf16)
            nc.sync.dma_start(w2[:], w2_d[:])
            b2 = consts.tile([1, NE], f32)
            nc.sync.dma_start(b2[:], b2_d[:])
            gb = consts.tile([NE, 1], f32)
            nc.sync.dma_start(gb[:], gb_d[:])
            ones = consts.tile([NE, 1], f32)
            nc.vector.memset(ones[:], 1.0)

            # gating logits: gl[e, t] = sum_d gw[d, e] * x[d, t]
            gl = ps_g.tile([NE, TOK], f32)
            for k in range(KT):
                nc.tensor.matmul(
                    gl[:], gw[:, k, :], xT[:, k, :], start=(k == 0), stop=(k == KT - 1)
                )
            expl = consts.tile([NE, TOK], f32)
            nc.scalar.activation(expl[:], gl[:], AF.Exp, bias=gb[:])

            # denominator: den[t] = sum_e expl[e, t]
            den = ps_g.tile([1, TOK], f32)
            nc.tensor.matmul(den[:], ones[:], expl[:], start=True, stop=True)
            rec = consts.tile([1, TOK], f32)
            nc.vector.reciprocal(rec[:], den[:])

            # flatten expl rows onto partition 0 so per-expert weighting is
            # a partition-0 elementwise op (cross-partition move via DMA)
            explf = consts.tile([1, NE * TOK], f32)
            for n in range(NE):
                nc.sync.dma_start(explf[0:1, n * TOK:(n + 1) * TOK], expl[n:n + 1, :])

            # weighted-sum accumulator on partition 0
            u = accp.tile([1, TOK], f32)
            nc.vector.memset(u[:], 0.0)

            for n in range(NE):
                if use_fp8:
                    w1s = wpool.tile(
                        [P, KT2, MT, P, 2] if drsw else [P, KT2, 2, MID], fp8
                    )
                    nc.sync.dma_start(w1s[:], w1_d[n])
                e_ps = ps_e.tile([1, TOK], f32)
                for mt in range(MT):
                    if not use_fp8:
                        # one [P, KT, 128] weight chunk per mid-tile: finer
                        # prefetch granularity, smaller SBUF footprint
                        w1c = wpool.tile([P, KT, P], bf16, bufs=8)
                        nc.sync.dma_start(w1c[:], w1_d[n, mt])
                    h_ps = ps_h.tile([P, TOK], f32)
                    if use_fp8:
                        for k2 in range(KT2):
                            nc.tensor.matmul(
                                h_ps[:],
                                w1s[:, k2, mt, :, :]
                                if drsw
                                else w1s[:, k2, :, mt * P:(mt + 1) * P],
                                xq[:, k2, :, :],
                                start=(k2 == 0),
                                stop=(k2 == KT2 - 1),
                                perf_mode=pm,
                            )
                    else:
                        for k in range(KT):
                            nc.tensor.matmul(
                                h_ps[:],
                                w1c[:, k, :],
                                xT[:, k, :],
                                start=(k == 0),
                                stop=(k == KT - 1),
                            )
                    h_sb = hpool.tile([P, TOK], bf16)
                    nc.scalar.activation(
                        h_sb[:],
                        h_ps[:],
                        AF.Relu,
                        bias=b1[:, n, mt:mt + 1],
                        scale=(1.0 / FP8_SCALE) if use_fp8 else 1.0,
                    )
                    nc.tensor.matmul(
                        e_ps[:],
                        w2[:, n, mt:mt + 1],
                        h_sb[:],
                        start=(mt == 0),
                        stop=(mt == MT - 1),
                        skip_group_check=True,
                    )
                # u += (e + b2[n]) * expl[n]
                tmp = small.tile([1, TOK], f32)
                nc.vector.scalar_tensor_tensor(
                    tmp[:],
                    e_ps[:],
                    b2[0:1, n:n + 1],
                    explf[0:1, n * TOK:(n + 1) * TOK],
                    ALU.add,
                    ALU.mult,
                )
                nc.vector.tensor_add(u[:], u[:], tmp[:])

            # out = sigmoid(u / den)
            s = small.tile([1, TOK], f32)
            nc.vector.tensor_mul(s[:], u[:], rec[:])
            o = small.tile([1, TOK], f32)
            nc.scalar.activation(o[:], s[:], AF.Sigmoid)
            nc.sync.dma_start(out_d[:], o[:])

    nc.compile()
    return nc


def _prep_inputs(x, w1, b1, w2, b2, gw, gb, use_fp8=USE_FP8):
    """Host-side shard + layout prep. Returns per-core in_maps."""
    if IMPL == "xs":
        return _prep_inputs_xs(x, w1, b1, w2, b2, gw, gb)
    bf = ml_dtypes.bfloat16
    import concourse.mybir as mybir

    fp8np = mybir.dt.np(mybir.dt.float8e4)
    xf = np.ascontiguousarray(np.asarray(x, np.float32)).reshape(T, D)
    # xT[core][p, k, t] = xf[core*TOK + t, k*P + p]
    xp = xf.reshape(NCORES, TOK, KT, P).transpose(0, 3, 2, 1).astype(bf)
    if use_fp8:
        # xq[core][p, k2, h, t] = xf[core*TOK + t, k2*256 + h*128 + p]
        xqp = (
            xf.reshape(NCORES, TOK, KT2, 2, P)
            .transpose(0, 4, 2, 3, 1)
            .astype(fp8np)
        )
        if FP8_MODE == "drsw":
            # A[n, k2, h, p, mt, m]
            a = (np.asarray(w1, np.float32) * FP8_SCALE).reshape(
                NE, KT2, 2, P, MT, P
            )
            # [n, p, k2, mt, j, i] with j = reversed m, i = row-half
            w1p = np.ascontiguousarray(
                a[:, :, :, :, :, ::-1].transpose(0, 3, 1, 4, 5, 2).astype(fp8np)
            )
        else:
            # w1q[n, p, k2, h, m] = FP8_SCALE * w1[n, k2*256 + h*128 + p, m]
            w1p = np.ascontiguousarray(
                (np.asarray(w1, np.float32) * FP8_SCALE)
                .reshape(NE, KT2, 2, P, MID)
                .transpose(0, 3, 1, 2, 4)
                .astype(fp8np)
            )
    else:
        # [NE, MT, P, KT, P]: w1p[n, mt, p, k, m] = w1[n, k*P+p, mt*P+m]
        w1p = np.ascontiguousarray(
            np.asarray(w1, np.float32)
            .reshape(NE, KT, P, MT, P)
            .transpose(0, 3, 2, 1, 4)
            .astype(bf)
        )
    gwp = np.ascontiguousarray(
        np.asarray(gw, np.float32).reshape(KT, P, NE).transpose(1, 0, 2).astype(bf)
    )
    b1p = np.ascontiguousarray(
        np.asarray(b1, np.float32).reshape(NE, MT, P).transpose(2, 0, 1)
    )
    w2p = np.ascontiguousarray(
        np.asarray(w2, np.float32).reshape(NE, MT, P).transpose(2, 0, 1).astype(bf)
    )
    b2p = np.asarray(b2, np.float32).reshape(1, NE)
    gbp = np.asarray(gb, np.float32).reshape(NE, 1)

    in_maps = []
    for c in range(NCORES):
        m = {
            "xT": np.ascontiguousarray(xp[c]),
            "w1": w1p,
            "gw": gwp,
            "b1": b1p,
            "w2": w2p,
            "b2": b2p,
            "gb": gbp,
        }
        if use_fp8:
            m["xq"] = np.ascontiguousarray(xqp[c])
        in_maps.append(m)
    return in_maps


def kernel(x, w1, b1, w2, b2, gw, gb):
    from concourse import bass_utils

    if "nc" not in _CACHE:
        _CACHE["nc"] = _build()
    nc = _CACHE["nc"]
    in_maps = _prep_inputs(x, w1, b1, w2, b2, gw, gb)
    res = bass_utils.run_bass_kernel_spmd(nc, in_maps, core_ids=list(range(NCORES)))
    if IMPL == "xs":
        # out[p, tb] -> core-local token tb*128 + p
        out = np.concatenate([r["out"].T.reshape(TOK) for r in res.results])
    else:
        out = np.concatenate([r["out"].reshape(TOK) for r in res.results])
    return out.reshape(B, NW).astype(np.float32)

